# revision 66
# baseline (speedup 1.0000x reference)
"""Causal self-attention (B=2, T=2048, C=1024, H=16) on 8 NeuronCores.

Sharding: data-parallel over batch (2) x tensor-parallel over heads
(4 groups of 4 heads). Each core computes q/k/v projections for its
head slice, causal attention for its 4 heads, and a partial c_proj
([2048,256] @ [256,1024]); the host sums the 4 partials per batch
(the "all-reduce") and folds the v/proj biases in at the end.

The reference uses scale = float32(C // (H ** -0.5)) = 4096.0 (exact
power of two), so logits are huge and softmax is near-one-hot; the
q/k projection and Q@K^T run in full fp32 to keep argmaxes faithful
(min top-2 scaled-logit gap on this data is ~0.2; fp16/bf16 logits
would flip rows). The value path (v projection, P@V, c_proj) runs in
fp16 — P entries are probabilities and v/out tolerances are ~5e-4.

K=64 fp32 matmuls waste half the PE array, so Q@K^T packs each HEAD
PAIR into one K=128 matmul with a block-diagonal stationary operand:
lhsT[0:64, 0:64] = qT_h0 block, lhsT[64:128, 64:128] = qT_h1 block,
zeros elsewhere; rhs rows 0:64 = kT_h0, 64:128 = kT_h1. Output rows
0:63 are h0's scores for 64 queries, rows 64:127 h1's — softmax
stats stay row-wise.

P@V avoids PE transposes entirely: after softmax, P is normalized
in-place (x 1/l, per-partition) and transposed by the DMA xbar
(SBUF->SBUF, on otherwise-idle DMA engines). Both head pairs of a
query block share one [128, 2*kpad] P tile so a single blocked
transpose issue covers them. The PV matmul uses the V tile as the
STATIONARY operand and P^T as the moving one, so the product lands
as out^T[hs, q] whose diagonal [64,64] quadrants are copied straight
into the aout [hs, token] layout c_proj consumes — no output
transposes either. klen is padded to a 128 multiple with causally
masked (=> zero-probability) columns so transpose and PV tiles are
uniform. Input loads are batched one DMA per tensor per chunk, and
c_proj is spread across the following chunk's attention units to
keep every queue short.

Per-core DRAM tensors:
  xTh  [1024, 2048] f16  x[b] transposed, high fp16 half
  xTl  [1024, 2048] f16  low half (x - xTh)
  wqk  [1024, 512]  f16  hi/lo: cols [q_h0|q_h1|q_h2|q_h3|k_h0|..|k_h3]
  bqk  [512, 1]     f32  matching bias layout
  wv   [1024, 256]  f16  v columns for the head group
  wp   [256, 1024]  f16  w_proj rows for the head group
  outT [1024, 2048] f16  partial output, transposed (the four fp16
                         partials are summed in fp32 on the host; the
                         ~5e-4 relative rounding this adds is well
                         inside the 2e-2 gate)
"""

import numpy as np

import concourse.bacc as bacc
import concourse.mybir as mybir
import concourse.tile as tile
from concourse.bass_utils import run_bass_kernel_spmd
from concourse.masks import make_causal_mask

f32 = mybir.dt.float32
f16 = mybir.dt.float16
AF = mybir.ActivationFunctionType
AX = mybir.AxisListType

B, T, C = 2, 2048, 1024
H, HS = 16, 64
NCORES = 8
HG = 4            # head groups (cores per batch)
NHL = H // HG     # local heads per core = 4
P = 128
KT = C // P       # 8 contraction tiles
CH = 512          # free-dim chunk
NT = T // CH      # 4 token chunks
QB64 = T // HS    # 32 query blocks of 64
SCALE = 4096.0    # float32(C // (H ** -0.5)) — faithful to source bug
MASK_VAL = -1e10

_CACHE = {}


def _build_program():
    nc = bacc.Bacc("TRN2", target_bir_lowering=False, debug=False,
                   num_devices=NCORES)
    xTh = nc.dram_tensor("xTh", [C, T], f16, kind="ExternalInput").ap()
    xTl = nc.dram_tensor("xTl", [C, T], f16, kind="ExternalInput").ap()
    wqkh = nc.dram_tensor("wqkh", [C, 2 * NHL * HS], f16, kind="ExternalInput").ap()
    wqkl = nc.dram_tensor("wqkl", [C, 2 * NHL * HS], f16, kind="ExternalInput").ap()
    bqk = nc.dram_tensor("bqk", [2 * NHL * HS, 1], f32, kind="ExternalInput").ap()
    wv = nc.dram_tensor("wv", [C, NHL * HS], f16, kind="ExternalInput").ap()
    wp = nc.dram_tensor("wp", [NHL * HS, C], f16, kind="ExternalInput").ap()
    outT = nc.dram_tensor("outT", [C, T], f16, kind="ExternalOutput").ap()

    with tile.TileContext(nc) as tc:
        with (
            tc.tile_pool(name="const", bufs=1) as const,
            tc.tile_pool(name="wts", bufs=1) as wts,
            tc.tile_pool(name="xin", bufs=2) as xin,
            tc.tile_pool(name="qksb", bufs=1) as qksb,
            tc.tile_pool(name="vsb", bufs=1) as vsb,
            tc.tile_pool(name="att", bufs=3) as att,
            tc.tile_pool(name="ptp", bufs=5) as ptp,
            tc.tile_pool(name="aout", bufs=1) as aout,
            tc.tile_pool(name="stage", bufs=2) as stage,
            tc.tile_pool(name="ps_big", bufs=5, space="PSUM") as ps_big,
            tc.tile_pool(name="ps_o", bufs=3, space="PSUM") as ps_o,
        ):
            # [128, 64] causal mask for 64-query diagonal blocks (both
            # partition halves identical: row r masks query r % 64) and
            # a [128, 128] variant whose right half is fully masked, for
            # query blocks whose klen pads up to the next 128 multiple.
            mask = const.tile([P, HS], f32, tag="mask")
            make_causal_mask(nc, mask[0:HS, :], mask_val=MASK_VAL)
            make_causal_mask(nc, mask[HS:P, :], mask_val=MASK_VAL)
            mask2 = const.tile([P, P], f32, tag="mask2")
            make_causal_mask(nc, mask2[0:HS, 0:HS], mask_val=MASK_VAL)
            make_causal_mask(nc, mask2[HS:P, 0:HS], mask_val=MASK_VAL)
            nc.gpsimd.memset(mask2[:, HS:P], MASK_VAL)

            # ---- batched loads: one DMA per tensor kt-half per chunk
            # (halved so the first projection matmuls of a chunk wait
            # on 0.5 MB, not 1 MB)
            def xload_one(src, tag, nt):
                t_ = xin.tile([P, KT * CH], f16, tag=tag, name=f"{tag}_{nt}")
                hk = KT // 2
                for h in range(2):
                    nc.sync.dma_start(
                        t_[:, h * hk * CH:(h + 1) * hk * CH]
                        .rearrange("p (k c) -> p k c", c=CH),
                        src.rearrange("(k p) t -> p k t", p=P)
                           [:, h * hk:(h + 1) * hk, nt * CH:(nt + 1) * CH])
                return t_

            def xload(nt):
                return (xload_one(xTh, "xh", nt), xload_one(xTl, "xl", nt))

            # chunk-0 loads ordered so the first (hh) projection pass's
            # operands arrive first
            WQ = 2 * NHL * HS  # 512
            hk = KT // 4
            wqkh_sb = wts.tile([P, KT * WQ], f16, tag="wqkh")
            x0h = xin.tile([P, KT * CH], f16, tag="xh", name="xh_0")
            # interleave wqkh/xh QUARTERS so the (cold) hh pass starts
            # after 0.5 MB of loads and trickles with arrivals
            for h in range(4):
                nc.sync.dma_start(
                    wqkh_sb[:, h * hk * WQ:(h + 1) * hk * WQ]
                    .rearrange("p (k c) -> p k c", c=WQ),
                    wqkh.rearrange("(k p) c -> p k c", p=P)
                        [:, h * hk:(h + 1) * hk, :])
                nc.sync.dma_start(
                    x0h[:, h * hk * CH:(h + 1) * hk * CH]
                    .rearrange("p (k c) -> p k c", c=CH),
                    xTh.rearrange("(k p) t -> p k t", p=P)
                       [:, h * hk:(h + 1) * hk, 0:CH])
            x0 = (x0h, xload_one(xTl, "xl", 0))
            wqkl_sb = wts.tile([P, KT * WQ], f16, tag="wqkl")
            for h in range(2):
                hk = KT // 2
                nc.sync.dma_start(
                    wqkl_sb[:, h * hk * WQ:(h + 1) * hk * WQ]
                    .rearrange("p (k c) -> p k c", c=WQ),
                    wqkl.rearrange("(k p) c -> p k c", p=P)
                        [:, h * hk:(h + 1) * hk, :])
            wv_sb = wts.tile([P, KT * NHL * HS], f16, tag="wv")
            nc.sync.dma_start(
                wv_sb[:].rearrange("p (k c) -> p k c", c=NHL * HS),
                wv.rearrange("(k p) c -> p k c", p=P))
            bqk_sb = wts.tile([P, 4], f32, tag="bqk")
            nc.sync.dma_start(
                bqk_sb[:].rearrange("p (m o) -> p m o", o=1),
                bqk.rearrange("(m p) o -> p m o", p=P))
            wp_sb = []

            def load_wp():
                for kt in range(NHL * HS // P):  # 2
                    t_ = wts.tile([P, C], f16, tag=f"wp{kt}", name=f"wp{kt}")
                    nc.sync.dma_start(t_[:], wp[kt * P:(kt + 1) * P, :])
                    wp_sb.append(t_)

            # ---- persistent activations ----------------------------
            # qbd[hp]: block-diagonal qT for head pair hp, [128, 2T]:
            #   col block 2g   rows 0:64   = qT_h(2hp),  tokens g*64..
            #   col block 2g+1 rows 64:128 = qT_h(2hp+1), tokens g*64..
            # kp[hp]: [128, T], rows 0:64 = kT_h(2hp), 64:128 = kT_h(2hp+1)
            qbd_h = [qksb.tile([P, 2 * T], f16, tag=f"qbdh{i}", name=f"qbdh{i}")
                     for i in range(2)]
            qbd_l = [qksb.tile([P, 2 * T], f16, tag=f"qbdl{i}", name=f"qbdl{i}")
                     for i in range(2)]
            kp_h = [qksb.tile([P, T], f16, tag=f"kph{i}", name=f"kph{i}")
                    for i in range(2)]
            kp_l = [qksb.tile([P, T], f16, tag=f"kpl{i}", name=f"kpl{i}")
                    for i in range(2)]
            v_sb = [vsb.tile([P, NHL * HS], f16, tag=f"v{i}", name=f"v{i}")
                    for i in range(T // P)]
            aout_sb = [aout.tile([P, T], f16, tag=f"at{i}", name=f"at{i}")
                       for i in range(2)]

            for t_ in qbd_h + qbd_l:
                nc.gpsimd.memset(t_[:], 0.0)

            # ---- phase 1: qkv projections (per token chunk) --------
            # pass_major (chunk 0): run the hh pass for all 4 column
            # tiles kt-outer first, so the PE starts as soon as the
            # first wqkh/xh load halves land instead of waiting for
            # the hl/lh operands. defer_v returns the v-projection as
            # thunks interleaved into the early (tiny) attention units.
            def phase1_chunk(nt, pass_major=False, defer_v=False):
                xh_, xl_ = x0 if nt == 0 else xload(nt)

                def xs(t_, kt):
                    return t_[:, kt * CH:(kt + 1) * CH]
                passes = [(wqkh_sb, xh_), (wqkh_sb, xl_), (wqkl_sb, xh_)]
                # q,k: exact-fp32 via fp16 hi/lo three-pass (hh, hl, lh)
                if pass_major:
                    pss = [ps_big.tile([P, CH], f32, tag="big",
                                       name=f"p1_{nt}_{mt}") for mt in range(4)]
                    for pi, (wsb, xsb) in enumerate(passes):
                        for kt in range(KT):
                            for mt in range(4):
                                nc.tensor.matmul(
                                    pss[mt][:],
                                    wsb[:, kt * WQ + mt * P:
                                        kt * WQ + (mt + 1) * P],
                                    xs(xsb, kt),
                                    start=(pi == 0 and kt == 0),
                                    stop=(pi == 2 and kt == KT - 1))
                for mt in range(4):
                    if pass_major:
                        ps = pss[mt]
                    else:
                        ps = ps_big.tile([P, CH], f32, tag="big")
                        for pi, (wsb, xsb) in enumerate(passes):
                            for kt in range(KT):
                                nc.tensor.matmul(
                                    ps[:],
                                    wsb[:, kt * WQ + mt * P:
                                        kt * WQ + (mt + 1) * P],
                                    xs(xsb, kt),
                                    start=(pi == 0 and kt == 0),
                                    stop=(pi == 2 and kt == KT - 1))
                    if mt < 2:  # q: scatter into block-diagonal hi/lo
                        hp = mt
                        dh = qbd_h[hp][:, nt * 2 * CH:(nt + 1) * 2 * CH].rearrange(
                            "p (b t c) -> p b t c", t=2, c=HS)
                        dl = qbd_l[hp][:, nt * 2 * CH:(nt + 1) * 2 * CH].rearrange(
                            "p (b t c) -> p b t c", t=2, c=HS)
                        sv = ps[:].rearrange("p (b c) -> p b c", c=HS)
                        for half, sl in ((0, slice(0, HS)), (1, slice(HS, P))):
                            nc.vector.tensor_scalar_add(
                                dh[sl, :, half, :], sv[sl],
                                bqk_sb[sl, mt:mt + 1])
                            nc.vector.scalar_tensor_tensor(
                                dl[sl, :, half, :], sv[sl],
                                bqk_sb[sl, mt:mt + 1],
                                dh[sl, :, half, :],
                                op0=mybir.AluOpType.add,
                                op1=mybir.AluOpType.subtract)
                    else:  # k: plain pair layout, hi then lo
                        hp = mt - 2
                        cs = slice(nt * CH, (nt + 1) * CH)
                        nc.scalar.activation(
                            kp_h[hp][:, cs], ps[:], AF.Identity,
                            bias=bqk_sb[:, mt:mt + 1])
                        nc.vector.scalar_tensor_tensor(
                            kp_l[hp][:, cs], ps[:], bqk_sb[:, mt:mt + 1],
                            kp_h[hp][:, cs],
                            op0=mybir.AluOpType.add,
                            op1=mybir.AluOpType.subtract)
                # v (natural layout, fp16): lhsT = xTh block, rhs = wv
                def vproj_tt(tt):
                    ps = ps_o.tile([P, CH], f32, tag="o")
                    for kt in range(KT):
                        nc.tensor.matmul(
                            ps[:, 0:NHL * HS],
                            xh_[:, kt * CH + tt * P:kt * CH + (tt + 1) * P],
                            wv_sb[:, kt * NHL * HS:(kt + 1) * NHL * HS],
                            start=(kt == 0), stop=(kt == KT - 1))
                    nc.vector.tensor_copy(v_sb[nt * (CH // P) + tt][:],
                                          ps[:, 0:NHL * HS])
                if defer_v:
                    return [lambda tt=tt: vproj_tt(tt)
                            for tt in range(CH // P)]
                for tt in range(CH // P):
                    vproj_tt(tt)

            # ---- phase 2: causal attention, software-pipelined -----
            # PE engine queues are in-order: emit S(qb) before PV(qb-2)
            # so the PE never idles on the exp/scale/transpose chain.
            # c_proj column tiles stage into one fp16 tile per token
            # range; a single batched DMA stores the whole range (fewer
            # DMAs -> fewer collisions with the serialized transposes)
            proj_state = {}

            def proj_cols(mts, lo, hi, tail=False):
                w = hi - lo
                if lo not in proj_state:
                    proj_state[lo] = [stage.tile([P, 8 * w], f16, tag="stage",
                                                 name=f"stg_{lo}"), 0]
                st, _ = proj_state[lo]
                for mt in mts:
                    ps = ps_big.tile([P, CH], f32, tag="big",
                                     name=f"proj_{mt}_{lo}")
                    for kt in range(2):
                        nc.tensor.matmul(
                            ps[:, 0:w], wp_sb[kt][:, mt * P:(mt + 1) * P],
                            aout_sb[kt][:, lo:hi],
                            start=(kt == 0), stop=(kt == 1))
                    # tail: alternate staging copies between ACT and
                    # DVE (both idle) and store in two halves so the
                    # final chain is shorter
                    if tail and mt % 2 == 1:
                        nc.vector.tensor_copy(st[:, mt * w:(mt + 1) * w],
                                              ps[:, 0:w])
                    else:
                        nc.scalar.activation(st[:, mt * w:(mt + 1) * w],
                                             ps[:, 0:w], AF.Copy)
                    proj_state[lo][1] += 1
                    if tail and proj_state[lo][1] == 4:
                        nc.sync.dma_start(
                            outT.rearrange("(m p) t -> p m t", p=P)
                                [:, 0:4, lo:hi],
                            st[:, :4 * w].rearrange("p (m c) -> p m c", c=w))
                    if tail and proj_state[lo][1] == 7:
                        nc.sync.dma_start(
                            outT.rearrange("(m p) t -> p m t", p=P)
                                [:, 4:7, lo:hi],
                            st[:, 4 * w:7 * w]
                            .rearrange("p (m c) -> p m c", c=w))
                if proj_state[lo][1] == 8:
                    src = st[:, 7 * w:].rearrange("p (m c) -> p m c", c=w) \
                        if tail else st[:].rearrange("p (m c) -> p m c", c=w)
                    dst = outT.rearrange("(m p) t -> p m t", p=P)
                    nc.sync.dma_start(
                        dst[:, 7:8, lo:hi] if tail else dst[:, :, lo:hi], src)

            def s_stage(un, hp):
                qb, kpad, nkb = un["qb"], un["kpad"], un["nkb"]
                klen = (qb + 1) * HS
                nch = (kpad + CH - 1) // CH
                p_sb = un["p_sb"]
                poff = hp * kpad
                mx = att.tile([P, 4], f32, tag=f"mx{hp}", name=f"mx_{qb}_{hp}")
                # diag (masked) chunk first: its mask->max chain overlaps
                # the remaining chunks' matmuls
                chunk_tiles = {}
                for kc in [nch - 1] + list(range(nch - 1)):
                    w = min(CH, kpad - kc * CH)
                    ps = ps_big.tile([P, CH], f32, tag="big",
                                     name=f"s_{qb}_{hp}_{kc}")
                    qs = slice(qb * P, (qb + 1) * P)
                    ks = slice(kc * CH, kc * CH + w)
                    for pi, (qt_, kt_) in enumerate((
                            (qbd_h[hp], kp_h[hp]), (qbd_h[hp], kp_l[hp]),
                            (qbd_l[hp], kp_h[hp]))):
                        nc.tensor.matmul(
                            ps[:, :w], qt_[:, qs], kt_[:, ks],
                            start=(pi == 0), stop=(pi == 2))
                    if kc == nch - 1:
                        if kpad > klen:  # padded: mask last 128 cols
                            off = kpad - kc * CH - P
                            nc.vector.tensor_add(
                                ps[:, off:off + P], ps[:, off:off + P],
                                mask2[:])
                        else:  # exact: mask the 64-col diagonal block
                            off = kpad - kc * CH - HS
                            nc.vector.tensor_add(
                                ps[:, off:off + HS], ps[:, off:off + HS],
                                mask[:])
                    nc.vector.reduce_max(mx[:, kc:kc + 1], ps[:, :w], axis=AX.X)
                    chunk_tiles[kc] = (ps, w)
                s_chunks = [chunk_tiles[kc] for kc in range(nch)]
                nm = att.tile([P, 1], f32, tag=f"nm{hp}", name=f"nm_{qb}_{hp}")
                nc.vector.reduce_max(nm[:], mx[:, :nch], axis=AX.X, negate=True)
                nmb = att.tile([P, 1], f32, tag=f"nmb{hp}", name=f"nmb_{qb}_{hp}")
                nc.vector.tensor_scalar_mul(nmb[:], nm[:], SCALE)
                lp = att.tile([P, 4], f32, tag=f"lp{hp}", name=f"lp_{qb}_{hp}")
                # exp the diag chunk first (it was computed first) so
                # its PSUM bank frees earliest
                for kc in [nch - 1] + list(range(nch - 1)):
                    ps, w = s_chunks[kc]
                    nc.scalar.activation(
                        p_sb[:, poff + kc * CH:poff + kc * CH + w], ps[:, :w],
                        AF.Exp, bias=nmb[:], scale=SCALE,
                        accum_out=lp[:, kc:kc + 1])
                l_ = att.tile([P, 1], f32, tag=f"l{hp}", name=f"l_{qb}_{hp}")
                nc.vector.reduce_sum(l_[:], lp[:, :nch], axis=AX.X)
                linv = att.tile([P, 1], f32, tag=f"li{hp}", name=f"li_{qb}_{hp}")
                nc.vector.reciprocal(linv[:], l_[:])
                un[f"linv{hp}"] = linv

            def t_stage(un):
                # normalize P in place (per-partition 1/l), then one
                # blocked DMA-xbar transpose per head pair:
                # pt[p, j, q] = P[q, j*128+p]. All transposes stay on
                # the SP HWDGE queue — concurrent xbar transposes from
                # the two HWDGE rings corrupt each other (observed as
                # localized bad query blocks). Per-hp issues halve the
                # granularity so PV(hp0) can start ~2us earlier.
                qb, kpad, p_sb = un["qb"], un["kpad"], un["p_sb"]
                pt = ptp.tile([P, 2 * T], f16, tag="pt", name=f"pt_{qb}")
                for hp in range(2):
                    nc.vector.tensor_scalar_mul(
                        p_sb[:, hp * kpad:(hp + 1) * kpad],
                        p_sb[:, hp * kpad:(hp + 1) * kpad], un[f"linv{hp}"][:])
                    nc.sync.dma_start(
                        pt[:, hp * kpad:(hp + 1) * kpad]
                        .rearrange("p (j q) -> p j q", q=P),
                        p_sb[:, hp * kpad:(hp + 1) * kpad], transpose=True)
                un["pt"] = pt

            def pv_stage(un, hp):
                qb, nkb, pt = un["qb"], un["nkb"], un["pt"]
                o_ps = ps_o.tile([P, CH], f32, tag="o", name=f"o_{qb}_{hp}")
                for kb in range(nkb):
                    j = hp * nkb + kb
                    nc.tensor.matmul(
                        o_ps[:, 0:P], v_sb[kb][:, hp * P:(hp + 1) * P],
                        pt[:, j * P:(j + 1) * P],
                        start=(kb == 0), stop=(kb == nkb - 1))
                # diagonal quadrants -> aout [hs, token] (h0 rows 0:64,
                # h1 rows 64:128), already normalized
                nc.vector.tensor_copy(
                    aout_sb[hp][0:HS, qb * HS:(qb + 1) * HS],
                    o_ps[0:HS, 0:HS])
                nc.vector.tensor_copy(
                    aout_sb[hp][HS:P, qb * HS:(qb + 1) * HS],
                    o_ps[HS:P, HS:P])

            units = {}
            nqb = QB64 // NT  # 8 query blocks unlocked per token chunk
            # proj emission schedule: chunk nt's c_proj (8 col tiles of
            # wp.T) is spread over iterations 3..7 of chunk nt+1; the
            # last chunk is split into two token halves (the first can
            # start once its query blocks are done) to shrink the tail.
            proj_sched = {4: (0, 1), 5: (2, 3), 6: (4, 5), 7: (6, 7)}
            for nt in range(NT):
                vthunks = phase1_chunk(nt, pass_major=(nt == 0),
                                       defer_v=(nt == 0))
                if nt == 0:
                    load_wp()
                for j, qb in enumerate(range(nt * nqb, (nt + 1) * nqb)):
                    klen = (qb + 1) * HS
                    nkb = (klen + P - 1) // P
                    un = {"qb": qb, "nkb": nkb, "kpad": nkb * P,
                          "p_sb": att.tile([P, 2 * T], f16, tag="P",
                                           name=f"p_{qb}")}
                    units[qb] = un
                    s_stage(un, 0)
                    s_stage(un, 1)
                    if qb >= 1:
                        t_stage(units[qb - 1])
                    if qb >= 4:
                        pv_stage(units[qb - 4], 0)
                        pv_stage(units[qb - 4], 1)
                        units.pop(qb - 4)
                    if nt == 0 and 1 <= j <= 4:
                        vthunks[j - 1]()  # deferred v-proj as PE filler
                    if nt > 0 and j in proj_sched:
                        proj_cols(proj_sched[j], (nt - 1) * CH, nt * CH)
                    if nt == NT - 1 and j == 7:
                        # first half of the last chunk: tokens
                        # [1536,1792] need query blocks 24..27, all
                        # assembled by pv(27) just above.
                        proj_cols(range(8), 3 * CH, 3 * CH + 4 * HS)
            # flush: pv(28/29) first so the PE isn't head-blocked on
            # t(31)'s (large) transpose issue + completion
            for qb in (28, 29):
                pv_stage(units[qb], 0)
                pv_stage(units[qb], 1)
            t_stage(units[31])
            for qb in (30, 31):
                pv_stage(units[qb], 0)
                pv_stage(units[qb], 1)
            proj_cols(range(8), 3 * CH + 4 * HS, T, tail=True)

    nc.compile()
    return nc


def _get_program():
    if "nc" not in _CACHE:
        _CACHE["nc"] = _build_program()
    return _CACHE["nc"]


def _per_core_inputs(x, w_attn, b_attn, w_proj):
    in_maps = []
    for core in range(NCORES):
        b = core // HG
        hg = core % HG
        xTc = np.ascontiguousarray(x[b].T.astype(np.float32))
        xh = xTc.astype(np.float16)
        xl = (xTc - xh.astype(np.float32)).astype(np.float16)
        qcols = []
        bcols = []
        # q head-pairs then k head-pairs: [q01 | q23 | k01 | k23]
        for off in (0, C):  # q then k
            for j in range(NHL):
                hgl = hg * NHL + j
                qcols.append(w_attn[:, off + hgl * HS: off + (hgl + 1) * HS])
                bcols.append(b_attn[off + hgl * HS: off + (hgl + 1) * HS])
        wqk_ = np.ascontiguousarray(
            np.concatenate(qcols, axis=1).astype(np.float32))
        wqkh_ = wqk_.astype(np.float16)
        wqkl_ = (wqk_ - wqkh_.astype(np.float32)).astype(np.float16)
        bqk_ = np.ascontiguousarray(
            np.concatenate(bcols)[:, None].astype(np.float32))
        wv_ = np.ascontiguousarray(
            w_attn[:, 2 * C + hg * NHL * HS: 2 * C + (hg + 1) * NHL * HS]
            .astype(np.float16))
        wp_ = np.ascontiguousarray(
            w_proj[hg * NHL * HS:(hg + 1) * NHL * HS, :].astype(np.float16))
        in_maps.append({"xTh": xh, "xTl": xl, "wqkh": wqkh_, "wqkl": wqkl_,
                        "bqk": bqk_, "wv": wv_, "wp": wp_})
    return in_maps


def run_sharded(x, w_attn, b_attn, w_proj, b_proj, trace=False, **kw):
    nc = _get_program()
    in_maps = _per_core_inputs(x, w_attn, b_attn, w_proj)
    res = run_bass_kernel_spmd(nc, in_maps, core_ids=list(range(NCORES)),
                               trace=trace, **kw)
    out = np.zeros((B, T, C), dtype=np.float32)
    for core in range(NCORES):
        out[core // HG] += res.results[core]["outT"].T
    corr = (b_attn[2 * C:].astype(np.float32) @ w_proj.astype(np.float32)
            + b_proj.astype(np.float32))
    out += corr[None, None, :]
    return out, res


def kernel(x, w_attn, b_attn, w_proj, b_proj):
    out, _ = run_sharded(np.asarray(x), np.asarray(w_attn), np.asarray(b_attn),
                         np.asarray(w_proj), np.asarray(b_proj))
    return out


# revision 68
# speedup vs baseline: 1.0660x; 1.0660x over previous
"""Causal self-attention (B=2, T=2048, C=1024, H=16) on 8 NeuronCores.

Sharding: data-parallel over batch (2) x tensor-parallel over heads
(4 groups of 4 heads). Each core computes q/k/v projections for its
head slice, causal attention for its 4 heads, and a partial c_proj
([2048,256] @ [256,1024]); the host sums the 4 partials per batch
(the "all-reduce") and folds the v/proj biases in at the end.

The reference uses scale = float32(C // (H ** -0.5)) = 4096.0 (exact
power of two), so logits are huge and softmax is near-one-hot; the
q/k projection and Q@K^T run in full fp32 to keep argmaxes faithful
(min top-2 scaled-logit gap on this data is ~0.2; fp16/bf16 logits
would flip rows). The value path (v projection, P@V, c_proj) runs in
fp16 — P entries are probabilities and v/out tolerances are ~5e-4.

K=64 fp32 matmuls waste half the PE array, so Q@K^T packs each HEAD
PAIR into one K=128 matmul with a block-diagonal stationary operand:
lhsT[0:64, 0:64] = qT_h0 block, lhsT[64:128, 64:128] = qT_h1 block,
zeros elsewhere; rhs rows 0:64 = kT_h0, 64:128 = kT_h1. Output rows
0:63 are h0's scores for 64 queries, rows 64:127 h1's — softmax
stats stay row-wise.

P@V avoids PE transposes entirely: after softmax, P is normalized
in-place (x 1/l, per-partition) and transposed by the DMA xbar
(SBUF->SBUF, on otherwise-idle DMA engines). Both head pairs of a
query block share one [128, 2*kpad] P tile so a single blocked
transpose issue covers them. The PV matmul uses the V tile as the
STATIONARY operand and P^T as the moving one, so the product lands
as out^T[hs, q] whose diagonal [64,64] quadrants are copied straight
into the aout [hs, token] layout c_proj consumes — no output
transposes either. klen is padded to a 128 multiple with causally
masked (=> zero-probability) columns so transpose and PV tiles are
uniform. Input loads are batched one DMA per tensor per chunk, and
c_proj is spread across the following chunk's attention units to
keep every queue short.

Per-core DRAM tensors:
  xTh  [1024, 2048] f16  x[b] transposed, high fp16 half
  xTl  [1024, 2048] f16  low half (x - xTh)
  wqk  [1024, 512]  f16  hi/lo: cols [q_h0|q_h1|q_h2|q_h3|k_h0|..|k_h3]
  bqk  [512, 1]     f32  matching bias layout
  wv   [1024, 256]  f16  v columns for the head group
  wp   [256, 1024]  f16  w_proj rows for the head group
  outT [1024, 2048] f16  partial output, transposed (the four fp16
                         partials are summed in fp32 on the host; the
                         ~5e-4 relative rounding this adds is well
                         inside the 2e-2 gate)
"""

import numpy as np

import concourse.bacc as bacc
import concourse.mybir as mybir
import concourse.tile as tile
from concourse.bass_utils import run_bass_kernel_spmd
from concourse.masks import make_causal_mask

f32 = mybir.dt.float32
f16 = mybir.dt.float16
AF = mybir.ActivationFunctionType
AX = mybir.AxisListType

B, T, C = 2, 2048, 1024
H, HS = 16, 64
NCORES = 8
HG = 4            # head groups (cores per batch)
NHL = H // HG     # local heads per core = 4
P = 128
KT = C // P       # 8 contraction tiles
CH = 512          # free-dim chunk
NT = T // CH      # 4 token chunks
QB64 = T // HS    # 32 query blocks of 64
SCALE = 4096.0    # float32(C // (H ** -0.5)) — faithful to source bug
MASK_VAL = -1e10

_CACHE = {}


def _build_program():
    nc = bacc.Bacc("TRN2", target_bir_lowering=False, debug=False,
                   num_devices=NCORES)
    xTh = nc.dram_tensor("xTh", [C, T], f16, kind="ExternalInput").ap()
    xTl = nc.dram_tensor("xTl", [C, T], f16, kind="ExternalInput").ap()
    wqkh = nc.dram_tensor("wqkh", [C, 2 * NHL * HS], f16, kind="ExternalInput").ap()
    wqkl = nc.dram_tensor("wqkl", [C, 2 * NHL * HS], f16, kind="ExternalInput").ap()
    bqk = nc.dram_tensor("bqk", [2 * NHL * HS, 1], f32, kind="ExternalInput").ap()
    wv = nc.dram_tensor("wv", [C, NHL * HS], f16, kind="ExternalInput").ap()
    wp = nc.dram_tensor("wp", [NHL * HS, C], f16, kind="ExternalInput").ap()
    outT = nc.dram_tensor("outT", [C, T], f16, kind="ExternalOutput").ap()

    with tile.TileContext(nc) as tc:
        with (
            tc.tile_pool(name="const", bufs=1) as const,
            tc.tile_pool(name="wts", bufs=1) as wts,
            tc.tile_pool(name="xin", bufs=2) as xin,
            tc.tile_pool(name="qksb", bufs=1) as qksb,
            tc.tile_pool(name="vsb", bufs=1) as vsb,
            tc.tile_pool(name="att", bufs=3) as att,
            tc.tile_pool(name="ptp", bufs=5) as ptp,
            tc.tile_pool(name="aout", bufs=1) as aout,
            tc.tile_pool(name="stage", bufs=2) as stage,
            tc.tile_pool(name="ps_big", bufs=6, space="PSUM") as ps_big,
            tc.tile_pool(name="ps_o", bufs=2, space="PSUM") as ps_o,
        ):
            # [128, 64] causal mask for 64-query diagonal blocks (both
            # partition halves identical: row r masks query r % 64) and
            # a [128, 128] variant whose right half is fully masked, for
            # query blocks whose klen pads up to the next 128 multiple.
            mask = const.tile([P, HS], f32, tag="mask")
            make_causal_mask(nc, mask[0:HS, :], mask_val=MASK_VAL)
            make_causal_mask(nc, mask[HS:P, :], mask_val=MASK_VAL)
            mask2 = const.tile([P, P], f32, tag="mask2")
            make_causal_mask(nc, mask2[0:HS, 0:HS], mask_val=MASK_VAL)
            make_causal_mask(nc, mask2[HS:P, 0:HS], mask_val=MASK_VAL)
            nc.gpsimd.memset(mask2[:, HS:P], MASK_VAL)

            # ---- batched loads: one DMA per tensor kt-half per chunk
            # (halved so the first projection matmuls of a chunk wait
            # on 0.5 MB, not 1 MB)
            def xload_one(src, tag, nt):
                t_ = xin.tile([P, KT * CH], f16, tag=tag, name=f"{tag}_{nt}")
                hk = KT // 2
                for h in range(2):
                    nc.sync.dma_start(
                        t_[:, h * hk * CH:(h + 1) * hk * CH]
                        .rearrange("p (k c) -> p k c", c=CH),
                        src.rearrange("(k p) t -> p k t", p=P)
                           [:, h * hk:(h + 1) * hk, nt * CH:(nt + 1) * CH])
                return t_

            def xload(nt):
                return (xload_one(xTh, "xh", nt), xload_one(xTl, "xl", nt))

            # chunk-0 loads ordered so the first (hh) projection pass's
            # operands arrive first
            WQ = 2 * NHL * HS  # 512
            hk = KT // 4
            wqkh_sb = wts.tile([P, KT * WQ], f16, tag="wqkh")
            x0h = xin.tile([P, KT * CH], f16, tag="xh", name="xh_0")
            # interleave wqkh/xh QUARTERS so the (cold) hh pass starts
            # after 0.5 MB of loads and trickles with arrivals
            for h in range(4):
                nc.sync.dma_start(
                    wqkh_sb[:, h * hk * WQ:(h + 1) * hk * WQ]
                    .rearrange("p (k c) -> p k c", c=WQ),
                    wqkh.rearrange("(k p) c -> p k c", p=P)
                        [:, h * hk:(h + 1) * hk, :])
                nc.sync.dma_start(
                    x0h[:, h * hk * CH:(h + 1) * hk * CH]
                    .rearrange("p (k c) -> p k c", c=CH),
                    xTh.rearrange("(k p) t -> p k t", p=P)
                       [:, h * hk:(h + 1) * hk, 0:CH])
            # xl/wqkl interleaved quarters too (pass 2 needs xl.q1
            # first, pass 3 wqkl.q1 — trickle in compute order)
            x0l = xin.tile([P, KT * CH], f16, tag="xl", name="xl_0")
            wqkl_sb = wts.tile([P, KT * WQ], f16, tag="wqkl")
            qk = KT // 4
            for h in range(4):
                nc.sync.dma_start(
                    x0l[:, h * qk * CH:(h + 1) * qk * CH]
                    .rearrange("p (k c) -> p k c", c=CH),
                    xTl.rearrange("(k p) t -> p k t", p=P)
                       [:, h * qk:(h + 1) * qk, 0:CH])
                nc.sync.dma_start(
                    wqkl_sb[:, h * qk * WQ:(h + 1) * qk * WQ]
                    .rearrange("p (k c) -> p k c", c=WQ),
                    wqkl.rearrange("(k p) c -> p k c", p=P)
                        [:, h * qk:(h + 1) * qk, :])
            x0 = (x0h, x0l)
            wv_sb = wts.tile([P, KT * NHL * HS], f16, tag="wv")
            nc.sync.dma_start(
                wv_sb[:].rearrange("p (k c) -> p k c", c=NHL * HS),
                wv.rearrange("(k p) c -> p k c", p=P))
            bqk_sb = wts.tile([P, 4], f32, tag="bqk")
            nc.sync.dma_start(
                bqk_sb[:].rearrange("p (m o) -> p m o", o=1),
                bqk.rearrange("(m p) o -> p m o", p=P))
            wp_sb = []

            def load_wp():
                for kt in range(NHL * HS // P):  # 2
                    t_ = wts.tile([P, C], f16, tag=f"wp{kt}", name=f"wp{kt}")
                    nc.sync.dma_start(t_[:], wp[kt * P:(kt + 1) * P, :])
                    wp_sb.append(t_)

            # ---- persistent activations ----------------------------
            # qbd[hp]: block-diagonal qT for head pair hp, [128, 2T]:
            #   col block 2g   rows 0:64   = qT_h(2hp),  tokens g*64..
            #   col block 2g+1 rows 64:128 = qT_h(2hp+1), tokens g*64..
            # kp[hp]: [128, T], rows 0:64 = kT_h(2hp), 64:128 = kT_h(2hp+1)
            qbd_h = [qksb.tile([P, 2 * T], f16, tag=f"qbdh{i}", name=f"qbdh{i}")
                     for i in range(2)]
            qbd_l = [qksb.tile([P, 2 * T], f16, tag=f"qbdl{i}", name=f"qbdl{i}")
                     for i in range(2)]
            kp_h = [qksb.tile([P, T], f16, tag=f"kph{i}", name=f"kph{i}")
                    for i in range(2)]
            kp_l = [qksb.tile([P, T], f16, tag=f"kpl{i}", name=f"kpl{i}")
                    for i in range(2)]
            v_sb = [vsb.tile([P, NHL * HS], f16, tag=f"v{i}", name=f"v{i}")
                    for i in range(T // P)]
            aout_sb = [aout.tile([P, T], f16, tag=f"at{i}", name=f"at{i}")
                       for i in range(2)]

            for t_ in qbd_h + qbd_l:
                nc.gpsimd.memset(t_[:], 0.0)

            # ---- phase 1: qkv projections (per token chunk) --------
            # pass_major (chunk 0): run the hh pass for all 4 column
            # tiles kt-outer first, so the PE starts as soon as the
            # first wqkh/xh load halves land instead of waiting for
            # the hl/lh operands. defer_v returns the v-projection as
            # thunks interleaved into the early (tiny) attention units.
            def phase1_chunk(nt, pass_major=False, defer_v=False):
                xh_, xl_ = x0 if nt == 0 else xload(nt)

                def xs(t_, kt):
                    return t_[:, kt * CH:(kt + 1) * CH]
                passes = [(wqkh_sb, xh_), (wqkh_sb, xl_), (wqkl_sb, xh_)]
                # q,k: exact-fp32 via fp16 hi/lo three-pass (hh, hl, lh)
                if pass_major:
                    pss = [ps_big.tile([P, CH], f32, tag="big",
                                       name=f"p1_{nt}_{mt}") for mt in range(4)]
                    for pi, (wsb, xsb) in enumerate(passes):
                        for kt in range(KT):
                            for mt in range(4):
                                nc.tensor.matmul(
                                    pss[mt][:],
                                    wsb[:, kt * WQ + mt * P:
                                        kt * WQ + (mt + 1) * P],
                                    xs(xsb, kt),
                                    start=(pi == 0 and kt == 0),
                                    stop=(pi == 2 and kt == KT - 1))
                for mt in range(4):
                    if pass_major:
                        ps = pss[mt]
                    else:
                        ps = ps_big.tile([P, CH], f32, tag="big")
                        for pi, (wsb, xsb) in enumerate(passes):
                            for kt in range(KT):
                                nc.tensor.matmul(
                                    ps[:],
                                    wsb[:, kt * WQ + mt * P:
                                        kt * WQ + (mt + 1) * P],
                                    xs(xsb, kt),
                                    start=(pi == 0 and kt == 0),
                                    stop=(pi == 2 and kt == KT - 1))
                    if mt < 2:  # q: scatter into block-diagonal hi/lo
                        hp = mt
                        dh = qbd_h[hp][:, nt * 2 * CH:(nt + 1) * 2 * CH].rearrange(
                            "p (b t c) -> p b t c", t=2, c=HS)
                        dl = qbd_l[hp][:, nt * 2 * CH:(nt + 1) * 2 * CH].rearrange(
                            "p (b t c) -> p b t c", t=2, c=HS)
                        sv = ps[:].rearrange("p (b c) -> p b c", c=HS)
                        for half, sl in ((0, slice(0, HS)), (1, slice(HS, P))):
                            nc.vector.tensor_scalar_add(
                                dh[sl, :, half, :], sv[sl],
                                bqk_sb[sl, mt:mt + 1])
                            nc.vector.scalar_tensor_tensor(
                                dl[sl, :, half, :], sv[sl],
                                bqk_sb[sl, mt:mt + 1],
                                dh[sl, :, half, :],
                                op0=mybir.AluOpType.add,
                                op1=mybir.AluOpType.subtract)
                    else:  # k: plain pair layout, hi then lo
                        hp = mt - 2
                        cs = slice(nt * CH, (nt + 1) * CH)
                        nc.scalar.activation(
                            kp_h[hp][:, cs], ps[:], AF.Identity,
                            bias=bqk_sb[:, mt:mt + 1])
                        nc.vector.scalar_tensor_tensor(
                            kp_l[hp][:, cs], ps[:], bqk_sb[:, mt:mt + 1],
                            kp_h[hp][:, cs],
                            op0=mybir.AluOpType.add,
                            op1=mybir.AluOpType.subtract)
                # v (natural layout, fp16): lhsT = xTh block, rhs = wv
                def vproj_tt(tt):
                    ps = ps_o.tile([P, CH], f32, tag="o")
                    for kt in range(KT):
                        nc.tensor.matmul(
                            ps[:, 0:NHL * HS],
                            xh_[:, kt * CH + tt * P:kt * CH + (tt + 1) * P],
                            wv_sb[:, kt * NHL * HS:(kt + 1) * NHL * HS],
                            start=(kt == 0), stop=(kt == KT - 1))
                    nc.vector.tensor_copy(v_sb[nt * (CH // P) + tt][:],
                                          ps[:, 0:NHL * HS])
                if defer_v:
                    return [lambda tt=tt: vproj_tt(tt)
                            for tt in range(CH // P)]
                for tt in range(CH // P):
                    vproj_tt(tt)

            # ---- phase 2: causal attention, software-pipelined -----
            # PE engine queues are in-order: emit S(qb) before PV(qb-2)
            # so the PE never idles on the exp/scale/transpose chain.
            # c_proj column tiles stage into one fp16 tile per token
            # range; a single batched DMA stores the whole range (fewer
            # DMAs -> fewer collisions with the serialized transposes)
            proj_state = {}

            def proj_cols(mts, lo, hi, tail=False):
                w = hi - lo
                if lo not in proj_state:
                    proj_state[lo] = [stage.tile([P, 8 * w], f16, tag="stage",
                                                 name=f"stg_{lo}"), 0]
                st, _ = proj_state[lo]
                for mt in mts:
                    ps = ps_big.tile([P, CH], f32, tag="big",
                                     name=f"proj_{mt}_{lo}")
                    for kt in range(2):
                        nc.tensor.matmul(
                            ps[:, 0:w], wp_sb[kt][:, mt * P:(mt + 1) * P],
                            aout_sb[kt][:, lo:hi],
                            start=(kt == 0), stop=(kt == 1))
                    # tail: alternate staging copies between ACT and
                    # DVE (both idle) and store in two halves so the
                    # final chain is shorter
                    if tail and mt % 2 == 1:
                        nc.vector.tensor_copy(st[:, mt * w:(mt + 1) * w],
                                              ps[:, 0:w])
                    else:
                        nc.scalar.activation(st[:, mt * w:(mt + 1) * w],
                                             ps[:, 0:w], AF.Copy)
                    proj_state[lo][1] += 1
                    if tail and proj_state[lo][1] == 4:
                        nc.sync.dma_start(
                            outT.rearrange("(m p) t -> p m t", p=P)
                                [:, 0:4, lo:hi],
                            st[:, :4 * w].rearrange("p (m c) -> p m c", c=w))
                    if tail and proj_state[lo][1] == 7:
                        nc.sync.dma_start(
                            outT.rearrange("(m p) t -> p m t", p=P)
                                [:, 4:7, lo:hi],
                            st[:, 4 * w:7 * w]
                            .rearrange("p (m c) -> p m c", c=w))
                if proj_state[lo][1] == 8:
                    src = st[:, 7 * w:].rearrange("p (m c) -> p m c", c=w) \
                        if tail else st[:].rearrange("p (m c) -> p m c", c=w)
                    dst = outT.rearrange("(m p) t -> p m t", p=P)
                    nc.sync.dma_start(
                        dst[:, 7:8, lo:hi] if tail else dst[:, :, lo:hi], src)

            def s_stage(un, hp):
                qb, kpad, nkb = un["qb"], un["kpad"], un["nkb"]
                klen = (qb + 1) * HS
                nch = (kpad + CH - 1) // CH
                p_sb = un["p_sb"]
                poff = hp * kpad
                mx = att.tile([P, 4], f32, tag=f"mx{hp}", name=f"mx_{qb}_{hp}")
                # diag (masked) chunk first: its mask->max chain overlaps
                # the remaining chunks' matmuls
                chunk_tiles = {}
                for kc in [nch - 1] + list(range(nch - 1)):
                    w = min(CH, kpad - kc * CH)
                    ps = ps_big.tile([P, CH], f32, tag="big",
                                     name=f"s_{qb}_{hp}_{kc}")
                    qs = slice(qb * P, (qb + 1) * P)
                    ks = slice(kc * CH, kc * CH + w)
                    for pi, (qt_, kt_) in enumerate((
                            (qbd_h[hp], kp_h[hp]), (qbd_h[hp], kp_l[hp]),
                            (qbd_l[hp], kp_h[hp]))):
                        nc.tensor.matmul(
                            ps[:, :w], qt_[:, qs], kt_[:, ks],
                            start=(pi == 0), stop=(pi == 2))
                    if kc == nch - 1:
                        if kpad > klen:  # padded: mask last 128 cols
                            off = kpad - kc * CH - P
                            nc.vector.tensor_add(
                                ps[:, off:off + P], ps[:, off:off + P],
                                mask2[:])
                        else:  # exact: mask the 64-col diagonal block
                            off = kpad - kc * CH - HS
                            nc.vector.tensor_add(
                                ps[:, off:off + HS], ps[:, off:off + HS],
                                mask[:])
                    nc.vector.reduce_max(mx[:, kc:kc + 1], ps[:, :w], axis=AX.X)
                    chunk_tiles[kc] = (ps, w)
                s_chunks = [chunk_tiles[kc] for kc in range(nch)]
                nm = att.tile([P, 1], f32, tag=f"nm{hp}", name=f"nm_{qb}_{hp}")
                nc.vector.reduce_max(nm[:], mx[:, :nch], axis=AX.X, negate=True)
                nmb = att.tile([P, 1], f32, tag=f"nmb{hp}", name=f"nmb_{qb}_{hp}")
                nc.vector.tensor_scalar_mul(nmb[:], nm[:], SCALE)
                lp = att.tile([P, 4], f32, tag=f"lp{hp}", name=f"lp_{qb}_{hp}")
                # exp the diag chunk first (it was computed first) so
                # its PSUM bank frees earliest
                for kc in [nch - 1] + list(range(nch - 1)):
                    ps, w = s_chunks[kc]
                    nc.scalar.activation(
                        p_sb[:, poff + kc * CH:poff + kc * CH + w], ps[:, :w],
                        AF.Exp, bias=nmb[:], scale=SCALE,
                        accum_out=lp[:, kc:kc + 1])
                l_ = att.tile([P, 1], f32, tag=f"l{hp}", name=f"l_{qb}_{hp}")
                nc.vector.reduce_sum(l_[:], lp[:, :nch], axis=AX.X)
                linv = att.tile([P, 1], f32, tag=f"li{hp}", name=f"li_{qb}_{hp}")
                nc.vector.reciprocal(linv[:], l_[:])
                un[f"linv{hp}"] = linv

            def t_stage(un):
                # normalize P in place (per-partition 1/l), then one
                # blocked DMA-xbar transpose per head pair:
                # pt[p, j, q] = P[q, j*128+p]. All transposes stay on
                # the SP HWDGE queue — concurrent xbar transposes from
                # the two HWDGE rings corrupt each other (observed as
                # localized bad query blocks). Per-hp issues halve the
                # granularity so PV(hp0) can start ~2us earlier.
                qb, kpad, p_sb = un["qb"], un["kpad"], un["p_sb"]
                pt = ptp.tile([P, 2 * T], f16, tag="pt", name=f"pt_{qb}")
                for hp in range(2):
                    nc.vector.tensor_scalar_mul(
                        p_sb[:, hp * kpad:(hp + 1) * kpad],
                        p_sb[:, hp * kpad:(hp + 1) * kpad], un[f"linv{hp}"][:])
                    nc.sync.dma_start(
                        pt[:, hp * kpad:(hp + 1) * kpad]
                        .rearrange("p (j q) -> p j q", q=P),
                        p_sb[:, hp * kpad:(hp + 1) * kpad], transpose=True)
                un["pt"] = pt

            def pv_stage(un, hp):
                qb, nkb, pt = un["qb"], un["nkb"], un["pt"]
                o_ps = ps_o.tile([P, CH], f32, tag="o", name=f"o_{qb}_{hp}")
                for kb in range(nkb):
                    j = hp * nkb + kb
                    nc.tensor.matmul(
                        o_ps[:, 0:P], v_sb[kb][:, hp * P:(hp + 1) * P],
                        pt[:, j * P:(j + 1) * P],
                        start=(kb == 0), stop=(kb == nkb - 1))
                # diagonal quadrants -> aout [hs, token] (h0 rows 0:64,
                # h1 rows 64:128), already normalized
                nc.vector.tensor_copy(
                    aout_sb[hp][0:HS, qb * HS:(qb + 1) * HS],
                    o_ps[0:HS, 0:HS])
                nc.vector.tensor_copy(
                    aout_sb[hp][HS:P, qb * HS:(qb + 1) * HS],
                    o_ps[HS:P, HS:P])

            units = {}
            nqb = QB64 // NT  # 8 query blocks unlocked per token chunk
            # proj emission schedule: chunk nt's c_proj (8 col tiles of
            # wp.T) is spread over iterations 3..7 of chunk nt+1; the
            # last chunk is split into two token halves (the first can
            # start once its query blocks are done) to shrink the tail.
            proj_sched = {4: (0, 1), 5: (2, 3), 6: (4, 5), 7: (6, 7)}
            for nt in range(NT):
                vthunks = phase1_chunk(nt, pass_major=(nt == 0),
                                       defer_v=(nt == 0))
                if nt == 0:
                    load_wp()
                for j, qb in enumerate(range(nt * nqb, (nt + 1) * nqb)):
                    klen = (qb + 1) * HS
                    nkb = (klen + P - 1) // P
                    un = {"qb": qb, "nkb": nkb, "kpad": nkb * P,
                          "p_sb": att.tile([P, 2 * T], f16, tag="P",
                                           name=f"p_{qb}")}
                    units[qb] = un
                    s_stage(un, 0)
                    s_stage(un, 1)
                    if qb >= 1:
                        t_stage(units[qb - 1])
                    if qb >= 4:
                        pv_stage(units[qb - 4], 0)
                        pv_stage(units[qb - 4], 1)
                        units.pop(qb - 4)
                    if nt == 0 and 1 <= j <= 4:
                        vthunks[j - 1]()  # deferred v-proj as PE filler
                    if nt > 0 and j in proj_sched:
                        proj_cols(proj_sched[j], (nt - 1) * CH, nt * CH)
                    if nt == NT - 1 and j == 7:
                        # first half of the last chunk: tokens
                        # [1536,1792] need query blocks 24..27, all
                        # assembled by pv(27) just above.
                        proj_cols(range(8), 3 * CH, 3 * CH + 4 * HS)
            # flush: pv(28/29) first so the PE isn't head-blocked on
            # t(31)'s (large) transpose issue + completion
            for qb in (28, 29):
                pv_stage(units[qb], 0)
                pv_stage(units[qb], 1)
            t_stage(units[31])
            for qb in (30, 31):
                pv_stage(units[qb], 0)
                pv_stage(units[qb], 1)
            proj_cols(range(8), 3 * CH + 4 * HS, T, tail=True)

    nc.compile()
    return nc


def _get_program():
    if "nc" not in _CACHE:
        _CACHE["nc"] = _build_program()
    return _CACHE["nc"]


def _per_core_inputs(x, w_attn, b_attn, w_proj):
    in_maps = []
    for core in range(NCORES):
        b = core // HG
        hg = core % HG
        xTc = np.ascontiguousarray(x[b].T.astype(np.float32))
        xh = xTc.astype(np.float16)
        xl = (xTc - xh.astype(np.float32)).astype(np.float16)
        qcols = []
        bcols = []
        # q head-pairs then k head-pairs: [q01 | q23 | k01 | k23]
        for off in (0, C):  # q then k
            for j in range(NHL):
                hgl = hg * NHL + j
                qcols.append(w_attn[:, off + hgl * HS: off + (hgl + 1) * HS])
                bcols.append(b_attn[off + hgl * HS: off + (hgl + 1) * HS])
        wqk_ = np.ascontiguousarray(
            np.concatenate(qcols, axis=1).astype(np.float32))
        wqkh_ = wqk_.astype(np.float16)
        wqkl_ = (wqk_ - wqkh_.astype(np.float32)).astype(np.float16)
        bqk_ = np.ascontiguousarray(
            np.concatenate(bcols)[:, None].astype(np.float32))
        wv_ = np.ascontiguousarray(
            w_attn[:, 2 * C + hg * NHL * HS: 2 * C + (hg + 1) * NHL * HS]
            .astype(np.float16))
        wp_ = np.ascontiguousarray(
            w_proj[hg * NHL * HS:(hg + 1) * NHL * HS, :].astype(np.float16))
        in_maps.append({"xTh": xh, "xTl": xl, "wqkh": wqkh_, "wqkl": wqkl_,
                        "bqk": bqk_, "wv": wv_, "wp": wp_})
    return in_maps


def run_sharded(x, w_attn, b_attn, w_proj, b_proj, trace=False, **kw):
    nc = _get_program()
    in_maps = _per_core_inputs(x, w_attn, b_attn, w_proj)
    res = run_bass_kernel_spmd(nc, in_maps, core_ids=list(range(NCORES)),
                               trace=trace, **kw)
    out = np.zeros((B, T, C), dtype=np.float32)
    for core in range(NCORES):
        out[core // HG] += res.results[core]["outT"].T
    corr = (b_attn[2 * C:].astype(np.float32) @ w_proj.astype(np.float32)
            + b_proj.astype(np.float32))
    out += corr[None, None, :]
    return out, res


def kernel(x, w_attn, b_attn, w_proj, b_proj):
    out, _ = run_sharded(np.asarray(x), np.asarray(w_attn), np.asarray(b_attn),
                         np.asarray(w_proj), np.asarray(b_proj))
    return out
